# revision 2
# baseline (speedup 1.0000x reference)
"""MoE (8 experts, top-2, cap-drop) Trainium2 kernel over 8 NeuronCores.

Strategy (expert-parallel, per sharding hint):
 - Router runs replicated on host (tiny: 134 MFLOP of the 344 GFLOP total) with
   the exact fp32 jax ops of the reference so top-2/capacity decisions match
   the oracle bit-for-bit; routing IS the sharding function here - it decides
   which token rows go to which expert shard.
 - Dispatch/shard: per expert e, gather its routed token rows (ascending token
   order, gate 0 on padding slots) and ship them transposed (D on partitions).
 - Each expert's FFN is split into 2 "units" along the hidden axis (2048 units
   each), giving 16 units; each core runs 2 units sequentially, bin-packed by
   routed token count so all cores get equal block counts. Per unit:
       ysT = W2h^T-chain( relu( W1h^T-chain( xT ) ) )
 - All matmuls in bf16 (fp32 PSUM accumulate): same PE rate as tf32
   (1 cycle/row) but half the DMA bytes and SBUF footprint, which lets BOTH
   units' weights be fully resident in SBUF and DMA'd up front - no weight
   streaming stalls at unit starts (the baseline lost ~60us there).
 - mm2's PSUM drain (copies to SBUF) alternates between DVE and the otherwise
   idle scalar engine, with 4 PSUM banks per matmul phase (warm-up shares
   mm1's rotating banks): measured ~170us/iter faster than DVE-only drain.
 - Combine/unshard: host sums the two half partial outputs per expert, applies
   the fp32 gates exactly as the reference (g[:,None] * h), and scatter-adds
   into y at the routed rows (row sets are disjoint within an expert).

Self-contained: shapes hardcoded for B=4, S=2048, D=1024, F=4096, E=8, top-2,
cap=2560, 8 cores.
"""

import sys

for _p in ("/opt/trn_rl_repo",):
    if _p not in sys.path:
        sys.path.append(_p)

import math
import os

import numpy as np
import ml_dtypes

BF16 = ml_dtypes.bfloat16

B, S, D, F, E = 4, 2048, 1024, 4096, 8
TOP_K = 2
CAP_FACTOR = 1.25
T = B * S                                   # 8192 tokens
CAP = max(math.ceil(T * TOP_K * CAP_FACTOR / E), 1)   # 2560
FH = F // 2                                 # 2048 hidden units per core
NCORES = 8
BLK = 512                                   # token block (matmul moving dim)
P = 128


def _route(xf: np.ndarray, Wr: np.ndarray):
    """Replicate the reference's routing bit-for-bit on jax-CPU.

    Returns per-expert (idx[CAP] int64 token ids, gate[CAP] f32, 0 on padding).
    """
    import jax
    import jax.numpy as jnp

    cpu = jax.devices("cpu")[0]
    with jax.default_device(cpu):
        xj = jnp.asarray(xf, dtype=jnp.float32)
        wr = jnp.asarray(Wr, dtype=jnp.float32)
        probs = jax.nn.softmax(xj.astype(jnp.float32) @ wr, axis=-1)
        topk_probs, topk_experts = jax.lax.top_k(probs, TOP_K)
        idxs, gates = [], []
        for e in range(E):
            mask = topk_experts == e
            gate = jnp.sum(jnp.where(mask, topk_probs, 0.0), axis=-1)
            has = jnp.any(mask, axis=-1)
            g_masked = jnp.where(has, gate, -jnp.inf)
            vals, idx = jax.lax.top_k(g_masked, CAP)
            g = jnp.where(jnp.isfinite(vals), vals, 0.0)
            idxs.append(np.asarray(idx, dtype=np.int64))
            gates.append(np.asarray(g, dtype=np.float32))
    return idxs, gates


_COMPILED = {}


def _blocks_of(q: int):
    """Token-block widths for a unit of q 256-col quarters: full 512-wide
    blocks plus one 256 tail when q is odd."""
    return [BLK] * (q // 2) + ([BLK // 2] if q % 2 else [])


def _build(qa: int, qb: int, reps: int = 1):
    """Compile the SPMD per-core program: two sequential units of a dense
    relu-MLP half, with qa / qb quarter-blocks (256 cols each) respectively.

    All weights (both units) are DMA'd at program start and stay resident in
    SBUF (bf16: 16 MB of the ~26 MB usable), so the PE never waits on weight
    streaming after the initial ~12us fill.
    """
    import concourse.bacc as bacc
    import concourse.mybir as mybir
    import concourse.tile as tile

    f32 = mybir.dt.float32
    bf16 = mybir.dt.bfloat16

    blocks = (_blocks_of(qa), _blocks_of(qb))
    nblk_a, nblk_b = len(blocks[0]), len(blocks[1])

    nc = bacc.Bacc("TRN2", target_bir_lowering=False, debug=False,
                   num_devices=NCORES)
    KD = D // P      # 8  k-chunks for matmul 1
    KF = FH // P     # 16 k-chunks for matmul 2
    # xg block-major: [block, k, p, j] so each SBUF tile is one contiguous DMA
    # (tail blocks are zero-padded to BLK on the host)
    xg = nc.dram_tensor("xg", [nblk_a + nblk_b, KD, P, BLK], bf16,
                        kind="ExternalInput")
    # w1 host-pretiled f-major: [u, f, p, k*P+m] = W1h[k*P+p, f*P+m]
    w1 = nc.dram_tensor("w1", [2, KF, P, D], bf16, kind="ExternalInput")
    # w2 host-pretiled d-major: [u, d, p, k2*P+m] = W2h[k2*P+p, d*P+m]
    w2 = nc.dram_tensor("w2", [2, KD, P, FH], bf16, kind="ExternalInput")
    ysa = nc.dram_tensor("ysa", [nblk_a, KD, P, BLK], f32, kind="ExternalOutput")
    ysb = nc.dram_tensor("ysb", [nblk_b, KD, P, BLK], f32, kind="ExternalOutput")
    ys_ts = (ysa, ysb)
    warm = nc.dram_tensor("warm", [P, BLK // 2], f32, kind="ExternalOutput")

    with tile.TileContext(nc) as tc:
        with (
            tc.tile_pool(name="w1p", bufs=1) as w1p,
            tc.tile_pool(name="w2p", bufs=1) as w2p,
            tc.tile_pool(name="xgp", bufs=3) as xgp,
            tc.tile_pool(name="htp", bufs=1) as htp,
            tc.tile_pool(name="outp", bufs=2) as outp,
            tc.tile_pool(name="warmp", bufs=1) as warmp,
            tc.tile_pool(name="ps1", bufs=4, space="PSUM") as ps1,
            tc.tile_pool(name="ps2", bufs=4, space="PSUM") as ps2,
        ):
            # PE warm-up: dummy matmuls on a memset tile keep the HAM
            # activity monitor busy (full 2.4 GHz clock) while the first
            # real xg/W1 DMAs land; they depend on no DMA and start at t0.
            wsrc = warmp.tile([P, BLK // 2], bf16, tag="warm_src")
            nc.vector.memset(wsrc[:], 0)
            wps = ps1.tile([P, BLK // 2], f32, tag="ps")
            for r in range(24):
                nc.tensor.matmul(wps[:], wsrc[:, :P], wsrc[:],
                                 start=(r == 0), stop=(r == 23))
            wout = warmp.tile([P, BLK // 2], f32, tag="warm_out")
            nc.vector.tensor_copy(wout[:], wps[:])
            nc.sync.dma_start(warm[:], wout[:])

            def body():
                # All weights end up resident in SBUF; DMA issue order is
                # chosen so the PE is never starved: xg block 0 first, then
                # unit-0 weights (needed immediately), while unit-1's weights
                # trickle in during unit-0's later blocks.
                w1sb = [[None] * KF for _ in range(2)]
                w2sb = [[None] * KD for _ in range(2)]

                def load_w1(u, fi):
                    t = w1p.tile([P, D], bf16, tag=f"w1_{u}_{fi}")
                    nc.sync.dma_start(t[:], w1[u, fi])
                    w1sb[u][fi] = t

                def load_w2(u, dd):
                    t = w2p.tile([P, FH], bf16, tag=f"w2_{u}_{dd}")
                    nc.sync.dma_start(t[:], w2[u, dd])
                    w2sb[u][dd] = t

                deferred = ([(load_w1, 1, fi) for fi in range(KF)]
                            + [(load_w2, 1, dd) for dd in range(KD)])
                nslots = max(1, len(blocks[0]) - 1)
                per_blk = (len(deferred) + nslots - 1) // nslots
                dpos = 0

                for u in range(2):
                    if u == 1:
                        # flush any unit-1 weight DMAs not yet issued
                        for fn, uu, ii in deferred[dpos:]:
                            fn(uu, ii)
                        dpos = len(deferred)
                    for b, bw in enumerate(blocks[u]):
                        bb = u * nblk_a + b
                        xgsb = []
                        for k in range(KD):
                            t = xgp.tile([P, BLK], bf16, tag=f"xg_{k}")
                            nc.sync.dma_start(t[:], xg[bb, k])
                            xgsb.append(t)
                        if u == 0 and b == 0:
                            # unit-0 weights, needed for this first block
                            for fi in range(KF):
                                load_w1(0, fi)
                            for dd in range(KD):
                                load_w2(0, dd)
                        elif u == 0:
                            # trickle unit-1 weights behind this block's xg
                            for fn, uu, ii in deferred[dpos:dpos + per_blk]:
                                fn(uu, ii)
                            dpos = min(len(deferred), dpos + per_blk)
                        hts = []
                        for fi in range(KF):
                            ps = ps1.tile([P, BLK], f32)
                            for k in range(KD):
                                nc.tensor.matmul(
                                    ps[:, :bw], w1sb[u][fi][:, k * P:(k + 1) * P],
                                    xgsb[k][:, :bw],
                                    start=(k == 0), stop=(k == KD - 1))
                            ht = htp.tile([P, BLK], bf16, tag=f"ht_{fi}")
                            if False:
                                # ACT takes every 4th relu; DVE (much faster
                                # per-op on HW) drains the rest so PSUM slots
                                # recycle at matmul pace.
                                nc.scalar.activation(
                                    ht[:, :bw], ps[:, :bw],
                                    mybir.ActivationFunctionType.Relu)
                            else:
                                nc.vector.tensor_scalar_max(
                                    ht[:, :bw], ps[:, :bw], 0.0)
                            hts.append(ht)
                        for d in range(KD):
                            ps_ = ps2.tile([P, BLK], f32)
                            for k2 in range(KF):
                                nc.tensor.matmul(
                                    ps_[:, :bw],
                                    w2sb[u][d][:, k2 * P:(k2 + 1) * P],
                                    hts[k2][:, :bw],
                                    start=(k2 == 0), stop=(k2 == KF - 1))
                            ob = outp.tile([P, BLK], f32)
                            if d % 2 == 1:
                                nc.scalar.activation(
                                    ob[:, :bw], ps_[:, :bw],
                                    mybir.ActivationFunctionType.Copy)
                            else:
                                nc.vector.tensor_copy(ob[:, :bw], ps_[:, :bw])
                            nc.sync.dma_start(ys_ts[u][b, d][:, :bw], ob[:, :bw])

            if reps == 1:
                body()
            else:
                # Bench-only variant: repeat the whole body on-device so the
                # per-iteration time dominates host dispatch overhead.
                with tc.For_i(0, reps, 1):
                    body()
    nc.compile()
    return nc


def _get_compiled(nblk_a: int, nblk_b: int):
    reps = int(os.environ.get("KERNEL_REPS", "1"))
    key = (nblk_a, nblk_b, reps)
    if key not in _COMPILED:
        _COMPILED[key] = _build(nblk_a, nblk_b, reps)
    return _COMPILED[key]


def kernel(x, Wr, W1, W2, _timing=None):
    from concourse.bass_utils import run_bass_kernel_spmd

    x = np.asarray(x, dtype=np.float32)
    Wr = np.asarray(Wr, dtype=np.float32)
    W1 = np.asarray(W1, dtype=np.float32)
    W2 = np.asarray(W2, dtype=np.float32)
    xf = x.reshape(T, D)

    # --- Host router (replicated, reference-exact) => sharding plan ---
    idxs, gates = _route(xf, Wr)
    counts = [int(np.count_nonzero(gates[e])) for e in range(E)]
    # unit sizes in 256-col quarters (tail block may be half-width)
    sizes = [max(1, math.ceil(c / (BLK // 2))) for c in counts]

    # --- Bin-pack the 16 (expert, half) units onto 8 cores, 2 units each.
    # Units of one expert share its size; with exactly 8 small + 8 large (or
    # all equal) units, every core gets an identical (small, large) shape.
    units = [(e, h) for e in range(E) for h in range(2)]
    usz = {u: sizes[u[0]] for u in units}
    distinct = sorted(set(usz.values()))
    if len(distinct) == 1:
        qa = qb = distinct[0]
    elif (len(distinct) == 2
          and sum(1 for u in units if usz[u] == distinct[0]) == E):
        qa, qb = distinct
    else:
        qa = qb = max(distinct)              # fallback: uniform padding
    small = [u for u in units if usz[u] <= qa][:E]
    large = [u for u in units if u not in small]
    assign = [(small[c], large[c]) for c in range(NCORES)]

    # --- Dispatch: gather + transpose routed rows per expert, cast bf16 ---
    xfT = np.ascontiguousarray(xf.T)                       # [D, T] f32
    cols = (qa * (BLK // 2), qb * (BLK // 2))
    pcols = (len(_blocks_of(qa)) * BLK, len(_blocks_of(qb)) * BLK)
    in_maps = []
    for c in range(NCORES):
        parts_xg, parts_w1, parts_w2 = [], [], []
        for s, (e, h) in enumerate(assign[c]):
            # block-major: [b, k, p, j], tail zero-padded to BLK
            xge = np.zeros((D, pcols[s]), dtype=np.float32)
            xge[:, :cols[s]] = xfT[:, idxs[e][:cols[s]]]
            parts_xg.append(
                xge.reshape(D // P, P, pcols[s] // BLK, BLK).transpose(2, 0, 1, 3))
            # f-major pretile: [f, p, k*P+m] = W1h[k*P+p, f*P+m]
            w1h = W1[e][:, h * FH:(h + 1) * FH]
            w1t = w1h.reshape(D // P, P, FH // P, P).transpose(2, 1, 0, 3)
            parts_w1.append(w1t.reshape(FH // P, P, D))
            # d-major pretile: [d, p, k2*P+m] = W2h[k2*P+p, d*P+m]
            w2h = W2[e][h * FH:(h + 1) * FH, :]
            w2t = w2h.reshape(FH // P, P, D // P, P).transpose(2, 1, 0, 3)
            parts_w2.append(w2t.reshape(D // P, P, FH))
        in_maps.append({
            "xg": np.ascontiguousarray(
                np.concatenate(parts_xg, axis=0)).astype(BF16),
            "w1": np.ascontiguousarray(np.stack(parts_w1)).astype(BF16),
            "w2": np.ascontiguousarray(np.stack(parts_w2)).astype(BF16),
        })

    # --- Device: 16 FFN half-units on 8 cores ---
    nc = _get_compiled(qa, qb)
    want_trace = _timing is not None and os.environ.get("KERNEL_TRACE", "1") == "1"
    tcores = [int(c) for c in os.environ.get(
        "KERNEL_TRACE_CORES", ",".join(map(str, range(NCORES)))).split(",")]
    try:
        res = run_bass_kernel_spmd(
            nc, in_maps, list(range(NCORES)),
            trace=want_trace,
            trace_cores=tcores if want_trace else None,
        )
    except ModuleNotFoundError:
        # NTFF profile hook unavailable in this environment: run untraced.
        res = run_bass_kernel_spmd(nc, in_maps, list(range(NCORES)))
    if _timing is not None:
        _timing["exec_time_ns"] = res.exec_time_ns
        _timing["results"] = res

    # --- Combine/unshard: sum halves per expert, gate, scatter-add (host) ---
    part = {}                                  # (e, h) -> [D, n_e]
    for c in range(NCORES):
        for s, (e, h) in enumerate(assign[c]):
            ys = res.results[c]["ysa" if s == 0 else "ysb"]   # [b, d, p, j]
            nb = ys.shape[0]
            part[(e, h)] = ys.transpose(1, 2, 0, 3).reshape(D, nb * BLK)
    y = np.zeros((T, D), dtype=np.float32)
    for e in range(E):
        n = counts[e]                          # only real (gate>0) slots
        hs = part[(e, 0)][:, :n] + part[(e, 1)][:, :n]       # [D, n]
        y[idxs[e][:n]] += gates[e][:n, None] * hs.T
    return y.reshape(B, S, D)



# revision 3
# speedup vs baseline: 114.1601x; 114.1601x over previous
"""MoE (8 experts, top-2, cap-drop) Trainium2 kernel over 8 NeuronCores.

Strategy v2 (expert-parallel, one full expert per core):
 - Router runs replicated on host with the exact fp32 jax ops of the
   reference so top-2/capacity decisions match the oracle bit-for-bit;
   routing IS the sharding function (it decides which token rows go to
   which expert core).
 - Gate folding: FFN(x) = W2^T relu(W1^T x) is positively homogeneous, and
   gates are softmax probs > 0, so g*FFN(x) = FFN(g*x). The host scales each
   gathered token column by its gate BEFORE the bf16 cast; the device then
   runs a pure dense FFN and the output needs no gating or masking at all
   (zero-gate padding columns produce exactly 0).
 - Dispatch/shard: per expert e (= core e), gather its routed token rows,
   scale by gate, ship transposed (D on partitions) in bf16, padded to the
   shared column count CMAX (all cores must run the same SPMD program).
 - Per core: ysT = W2^T-chain( relu( W1^T-chain( xT ) ) ) over column
   blocks of 512 (tail to CMAX%512); all matmuls bf16 with fp32 PSUM.
   All weights (16.8 MB bf16) are DMA'd up front and stay SBUF-resident.
 - Combine/unshard: output ships back as bf16 [8,128,CMAX]; host
   scatter-adds the first count_e (real) columns into y in f32.

vs v1 (two half-expert units per core, f32 partial outputs): same FLOPs,
but per-core host<->device traffic drops 45.2 MB -> ~25.7 MB (no f32
partials, no half-duplication of outputs, exact-count padding).

Self-contained: shapes hardcoded for B=4, S=2048, D=1024, F=4096, E=8,
top-2, cap=2560, 8 cores.
"""

import sys

for _p in ("/opt/trn_rl_repo",):
    if _p not in sys.path:
        sys.path.append(_p)

import math
import os

import numpy as np
import ml_dtypes

BF16 = ml_dtypes.bfloat16

B, S, D, F, E = 4, 2048, 1024, 4096, 8
TOP_K = 2
CAP_FACTOR = 1.25
T = B * S                                   # 8192 tokens
CAP = max(math.ceil(T * TOP_K * CAP_FACTOR / E), 1)   # 2560
NCORES = 8
BLK = 512                                   # token block (matmul moving dim)
P = 128
KD = D // P                                 # 8 k-chunks (embed)
KF = F // P                                 # 32 k-chunks (ffn)


def _route(xf: np.ndarray, Wr: np.ndarray):
    """Replicate the reference's routing bit-for-bit on jax-CPU.

    Returns per-expert (idx[CAP] int64 token ids, gate[CAP] f32, 0 on padding).
    """
    import jax
    import jax.numpy as jnp

    cpu = jax.devices("cpu")[0]
    with jax.default_device(cpu):
        xj = jnp.asarray(xf, dtype=jnp.float32)
        wr = jnp.asarray(Wr, dtype=jnp.float32)
        probs = jax.nn.softmax(xj.astype(jnp.float32) @ wr, axis=-1)
        topk_probs, topk_experts = jax.lax.top_k(probs, TOP_K)
        idxs, gates = [], []
        for e in range(E):
            mask = topk_experts == e
            gate = jnp.sum(jnp.where(mask, topk_probs, 0.0), axis=-1)
            has = jnp.any(mask, axis=-1)
            g_masked = jnp.where(has, gate, -jnp.inf)
            vals, idx = jax.lax.top_k(g_masked, CAP)
            g = jnp.where(jnp.isfinite(vals), vals, 0.0)
            idxs.append(np.asarray(idx, dtype=np.int64))
            gates.append(np.asarray(g, dtype=np.float32))
    return idxs, gates


_COMPILED = {}


def _blocks_of(cmax: int):
    bs = [BLK] * (cmax // BLK)
    if cmax % BLK:
        bs.append(cmax % BLK)
    return bs


def _build(cmax: int, reps: int = 1):
    """Compile the SPMD per-core program: one dense relu-MLP (full expert)
    over cmax token columns."""
    import concourse.bacc as bacc
    import concourse.mybir as mybir
    import concourse.tile as tile

    f32 = mybir.dt.float32
    bf16 = mybir.dt.bfloat16

    blocks = _blocks_of(cmax)

    nc = bacc.Bacc("TRN2", target_bir_lowering=False, debug=False,
                   num_devices=NCORES)
    xg = nc.dram_tensor("xg", [KD, P, cmax], bf16, kind="ExternalInput")
    # w1 host-pretiled f-major: [f, p, k*P+m] = W1[e][k*P+p, f*P+m]
    w1 = nc.dram_tensor("w1", [KF, P, D], bf16, kind="ExternalInput")
    # w2 host-pretiled d-major: [d, p, k2*P+m] = W2[e][k2*P+p, d*P+m]
    w2 = nc.dram_tensor("w2", [KD, P, F], bf16, kind="ExternalInput")
    ys = nc.dram_tensor("ys", [KD, P, cmax], bf16, kind="ExternalOutput")
    warm = nc.dram_tensor("warm", [P, BLK // 2], f32, kind="ExternalOutput")

    with tile.TileContext(nc) as tc:
        with (
            tc.tile_pool(name="w1p", bufs=1) as w1p,
            tc.tile_pool(name="w2p", bufs=1) as w2p,
            tc.tile_pool(name="xg0p", bufs=1) as xg0p,
            tc.tile_pool(name="xgp", bufs=2) as xgp,
            tc.tile_pool(name="htp", bufs=1) as htp,
            tc.tile_pool(name="outp", bufs=2) as outp,
            tc.tile_pool(name="warmp", bufs=1) as warmp,
            tc.tile_pool(name="ps1", bufs=4, space="PSUM") as ps1,
            tc.tile_pool(name="ps2", bufs=4, space="PSUM") as ps2,
        ):
            # PE warm-up: dummy matmuls on a memset tile keep the HAM
            # activity monitor busy (full 2.4 GHz clock) while the first
            # real xg/W1 DMAs land; they depend on no DMA and start at t0.
            wsrc = warmp.tile([P, BLK // 2], bf16, tag="warm_src")
            nc.vector.memset(wsrc[:], 0)
            wps = ps1.tile([P, BLK // 2], f32, tag="ps")
            for r in range(24):
                nc.tensor.matmul(wps[:], wsrc[:, :P], wsrc[:],
                                 start=(r == 0), stop=(r == 23))
            wout = warmp.tile([P, BLK // 2], f32, tag="warm_out")
            nc.vector.tensor_copy(wout[:], wps[:])
            nc.sync.dma_start(warm[:], wout[:])

            # Prelude DMA, issued once before the (optional) rep loop:
            # block-0 xg tiles first (the first matmul's operand — heads of
            # the round-robin DMA lanes), then w1 tiles (consumed first by
            # mm1, ~1.7us apart), then w2 (mm2 starts ~55us in; the full
            # 16.8 MB weight fill is ~47us). Weights and block-0 xg stay
            # SBUF-resident across reps.
            xg0sb = []
            for k in range(KD):
                t = xg0p.tile([P, BLK], bf16, tag=f"xg0_{k}")
                nc.sync.dma_start(t[:, :blocks[0]], xg[k, :, :blocks[0]])
                xg0sb.append(t)
            w1sb = [None] * KF
            w2sb = [None] * KD
            for fi in range(KF):
                t = w1p.tile([P, D], bf16, tag=f"w1_{fi}")
                nc.sync.dma_start(t[:], w1[fi])
                w1sb[fi] = t
            for dd in range(KD):
                t = w2p.tile([P, F], bf16, tag=f"w2_{dd}")
                nc.sync.dma_start(t[:], w2[dd])
                w2sb[dd] = t

            def body():
                for b, bw in enumerate(blocks):
                    c0 = b * BLK
                    if b == 0:
                        xgsb = xg0sb
                    else:
                        xgsb = []
                        for k in range(KD):
                            t = xgp.tile([P, BLK], bf16, tag=f"xg_{k}")
                            nc.sync.dma_start(t[:, :bw], xg[k, :, c0:c0 + bw])
                            xgsb.append(t)
                    hts = []
                    for fi in range(KF):
                        ps = ps1.tile([P, BLK], f32)
                        for k in range(KD):
                            nc.tensor.matmul(
                                ps[:, :bw], w1sb[fi][:, k * P:(k + 1) * P],
                                xgsb[k][:, :bw],
                                start=(k == 0), stop=(k == KD - 1))
                        ht = htp.tile([P, BLK], bf16, tag=f"ht_{fi}")
                        nc.vector.tensor_scalar_max(
                            ht[:, :bw], ps[:, :bw], 0.0)
                        hts.append(ht)
                    for d in range(KD):
                        ps_ = ps2.tile([P, BLK], f32)
                        for k2 in range(KF):
                            nc.tensor.matmul(
                                ps_[:, :bw],
                                w2sb[d][:, k2 * P:(k2 + 1) * P],
                                hts[k2][:, :bw],
                                start=(k2 == 0), stop=(k2 == KF - 1))
                        ob = outp.tile([P, BLK], bf16)
                        if d % 2 == 1:
                            nc.scalar.activation(
                                ob[:, :bw], ps_[:, :bw],
                                mybir.ActivationFunctionType.Copy)
                        else:
                            nc.vector.tensor_copy(ob[:, :bw], ps_[:, :bw])
                        nc.sync.dma_start(ys[d, :, c0:c0 + bw], ob[:, :bw])

            if reps == 1:
                body()
            else:
                # Bench-only variant: repeat the whole body on-device so the
                # per-iteration time dominates host dispatch overhead.
                with tc.For_i(0, reps, 1):
                    body()
    nc.compile()
    return nc


def _get_compiled(cmax: int):
    reps = int(os.environ.get("KERNEL_REPS", "1"))
    key = (cmax, reps)
    if key not in _COMPILED:
        _COMPILED[key] = _build(cmax, reps)
    return _COMPILED[key]


def kernel(x, Wr, W1, W2, _timing=None):
    from concourse.bass_utils import run_bass_kernel_spmd

    x = np.asarray(x, dtype=np.float32)
    Wr = np.asarray(Wr, dtype=np.float32)
    W1 = np.asarray(W1, dtype=np.float32)
    W2 = np.asarray(W2, dtype=np.float32)
    xf = x.reshape(T, D)

    # --- Host router (replicated, reference-exact) => sharding plan ---
    idxs, gates = _route(xf, Wr)
    counts = [int(np.count_nonzero(gates[e])) for e in range(E)]
    cmax = min(CAP, max(P, max(math.ceil(c / P) * P for c in counts)))

    # --- Dispatch: gather routed rows per expert, fold gate, cast bf16 ---
    xfT = np.ascontiguousarray(xf.T)                       # [D, T] f32
    in_maps = []
    for e in range(E):
        n = counts[e]
        xge = np.zeros((D, cmax), dtype=np.float32)
        xge[:, :n] = xfT[:, idxs[e][:n]] * gates[e][:n][None, :]
        w1t = W1[e].reshape(KD, P, KF, P).transpose(2, 1, 0, 3)
        w2t = W2[e].reshape(KF, P, KD, P).transpose(2, 1, 0, 3)
        in_maps.append({
            "xg": np.ascontiguousarray(
                xge.reshape(KD, P, cmax)).astype(BF16),
            "w1": np.ascontiguousarray(w1t.reshape(KF, P, D)).astype(BF16),
            "w2": np.ascontiguousarray(w2t.reshape(KD, P, F)).astype(BF16),
        })

    # --- Device: 8 expert FFNs on 8 cores ---
    nc = _get_compiled(cmax)
    want_trace = _timing is not None and os.environ.get("KERNEL_TRACE", "1") == "1"
    tcores = [int(c) for c in os.environ.get(
        "KERNEL_TRACE_CORES", ",".join(map(str, range(NCORES)))).split(",")]
    try:
        res = run_bass_kernel_spmd(
            nc, in_maps, list(range(NCORES)),
            trace=want_trace,
            trace_cores=tcores if want_trace else None,
        )
    except ModuleNotFoundError:
        # NTFF profile hook unavailable in this environment: run untraced.
        res = run_bass_kernel_spmd(nc, in_maps, list(range(NCORES)))
    if _timing is not None:
        _timing["exec_time_ns"] = res.exec_time_ns
        _timing["results"] = res

    # --- Combine/unshard: scatter-add gated outputs into y (host, f32) ---
    y = np.zeros((T, D), dtype=np.float32)
    for e in range(E):
        n = counts[e]
        yse = res.results[e]["ys"]                         # [KD, P, cmax] bf16
        y[idxs[e][:n]] += yse.reshape(D, cmax)[:, :n].T.astype(np.float32)
    return y.reshape(B, S, D)


# revision 4
# speedup vs baseline: 114.5431x; 1.0034x over previous
"""MoE (8 experts, top-2, cap-drop) Trainium2 kernel over 8 NeuronCores.

Strategy v2 (expert-parallel, one full expert per core):
 - Router runs replicated on host with the exact fp32 jax ops of the
   reference so top-2/capacity decisions match the oracle bit-for-bit;
   routing IS the sharding function (it decides which token rows go to
   which expert core).
 - Gate folding: FFN(x) = W2^T relu(W1^T x) is positively homogeneous, and
   gates are softmax probs > 0, so g*FFN(x) = FFN(g*x). The host scales each
   gathered token column by its gate BEFORE the bf16 cast; the device then
   runs a pure dense FFN and the output needs no gating or masking at all
   (zero-gate padding columns produce exactly 0).
 - Dispatch/shard: per expert e (= core e), gather its routed token rows,
   scale by gate, ship transposed (D on partitions) in bf16, padded to the
   shared column count CMAX (all cores must run the same SPMD program).
 - Per core: ysT = W2^T-chain( relu( W1^T-chain( xT ) ) ) over column
   blocks of 512 (tail to CMAX%512); all matmuls bf16 with fp32 PSUM.
   All weights (16.8 MB bf16) are DMA'd up front and stay SBUF-resident.
 - Combine/unshard: output ships back as bf16 [8,128,CMAX]; host
   scatter-adds the first count_e (real) columns into y in f32.

vs v1 (two half-expert units per core, f32 partial outputs): same FLOPs,
but per-core host<->device traffic drops 45.2 MB -> ~25.7 MB (no f32
partials, no half-duplication of outputs, exact-count padding).

Self-contained: shapes hardcoded for B=4, S=2048, D=1024, F=4096, E=8,
top-2, cap=2560, 8 cores.
"""

import sys

for _p in ("/opt/trn_rl_repo",):
    if _p not in sys.path:
        sys.path.append(_p)

import math
import os

import numpy as np
import ml_dtypes

BF16 = ml_dtypes.bfloat16

B, S, D, F, E = 4, 2048, 1024, 4096, 8
TOP_K = 2
CAP_FACTOR = 1.25
T = B * S                                   # 8192 tokens
CAP = max(math.ceil(T * TOP_K * CAP_FACTOR / E), 1)   # 2560
NCORES = 8
BLK = 512                                   # token block (matmul moving dim)
P = 128
KD = D // P                                 # 8 k-chunks (embed)
KF = F // P                                 # 32 k-chunks (ffn)


def _route(xf: np.ndarray, Wr: np.ndarray):
    """Replicate the reference's routing bit-for-bit on jax-CPU.

    Returns per-expert (idx[CAP] int64 token ids, gate[CAP] f32, 0 on padding).
    """
    import jax
    import jax.numpy as jnp

    cpu = jax.devices("cpu")[0]
    with jax.default_device(cpu):
        xj = jnp.asarray(xf, dtype=jnp.float32)
        wr = jnp.asarray(Wr, dtype=jnp.float32)
        probs = jax.nn.softmax(xj.astype(jnp.float32) @ wr, axis=-1)
        topk_probs, topk_experts = jax.lax.top_k(probs, TOP_K)
        idxs, gates = [], []
        for e in range(E):
            mask = topk_experts == e
            gate = jnp.sum(jnp.where(mask, topk_probs, 0.0), axis=-1)
            has = jnp.any(mask, axis=-1)
            g_masked = jnp.where(has, gate, -jnp.inf)
            vals, idx = jax.lax.top_k(g_masked, CAP)
            g = jnp.where(jnp.isfinite(vals), vals, 0.0)
            idxs.append(np.asarray(idx, dtype=np.int64))
            gates.append(np.asarray(g, dtype=np.float32))
    return idxs, gates


_COMPILED = {}


def _blocks_of(cmax: int):
    bs = [BLK] * (cmax // BLK)
    if cmax % BLK:
        bs.append(cmax % BLK)
    return bs


def _build(cmax: int, reps: int = 1):
    """Compile the SPMD per-core program: one dense relu-MLP (full expert)
    over cmax token columns."""
    import concourse.bacc as bacc
    import concourse.mybir as mybir
    import concourse.tile as tile

    f32 = mybir.dt.float32
    bf16 = mybir.dt.bfloat16

    blocks = _blocks_of(cmax)

    nc = bacc.Bacc("TRN2", target_bir_lowering=False, debug=False,
                   num_devices=NCORES)
    xg = nc.dram_tensor("xg", [KD, P, cmax], bf16, kind="ExternalInput")
    # w1 host-pretiled f-major: [f, p, k*P+m] = W1[e][k*P+p, f*P+m]
    w1 = nc.dram_tensor("w1", [KF, P, D], bf16, kind="ExternalInput")
    # w2 host-pretiled d-major: [d, p, k2*P+m] = W2[e][k2*P+p, d*P+m]
    w2 = nc.dram_tensor("w2", [KD, P, F], bf16, kind="ExternalInput")
    ys = nc.dram_tensor("ys", [KD, P, cmax], bf16, kind="ExternalOutput")
    warm = nc.dram_tensor("warm", [P, BLK // 2], f32, kind="ExternalOutput")

    with tile.TileContext(nc) as tc:
        with (
            tc.tile_pool(name="w1p", bufs=1) as w1p,
            tc.tile_pool(name="w2p", bufs=1) as w2p,
            tc.tile_pool(name="xg0p", bufs=1) as xg0p,
            tc.tile_pool(name="xgp", bufs=2) as xgp,
            tc.tile_pool(name="htp", bufs=1) as htp,
            tc.tile_pool(name="outp", bufs=2) as outp,
            tc.tile_pool(name="warmp", bufs=1) as warmp,
            tc.tile_pool(name="ps1", bufs=4, space="PSUM") as ps1,
            tc.tile_pool(name="ps2", bufs=4, space="PSUM") as ps2,
        ):
            # PE warm-up: dummy matmuls on a memset tile keep the HAM
            # activity monitor busy (full 2.4 GHz clock) while the first
            # real xg/W1 DMAs land; they depend on no DMA and start at t0.
            wsrc = warmp.tile([P, BLK // 2], bf16, tag="warm_src")
            nc.vector.memset(wsrc[:], 0)
            wps = ps1.tile([P, BLK // 2], f32, tag="ps")
            for r in range(12):
                nc.tensor.matmul(wps[:], wsrc[:, :P], wsrc[:],
                                 start=(r == 0), stop=(r == 11))
            wout = warmp.tile([P, BLK // 2], f32, tag="warm_out")
            nc.vector.tensor_copy(wout[:], wps[:])
            nc.scalar.dma_start(warm[:], wout[:])

            # Prelude DMA, issued once before the (optional) rep loop:
            # block-0 xg tiles first (the first matmul's operand — heads of
            # the round-robin DMA lanes), then w1 tiles (consumed first by
            # mm1, ~1.7us apart), then w2 (mm2 starts ~55us in; the full
            # 16.8 MB weight fill is ~47us). Weights and block-0 xg stay
            # SBUF-resident across reps.
            xg0sb = []
            for k in range(KD):
                t = xg0p.tile([P, BLK], bf16, tag=f"xg0_{k}")
                nc.sync.dma_start(t[:, :blocks[0]], xg[k, :, :blocks[0]])
                xg0sb.append(t)
            w1sb = [None] * KF
            w2sb = [None] * KD
            for fi in range(KF):
                t = w1p.tile([P, D], bf16, tag=f"w1_{fi}")
                nc.scalar.dma_start(t[:], w1[fi])
                w1sb[fi] = t
            for dd in range(KD):
                t = w2p.tile([P, F], bf16, tag=f"w2_{dd}")
                nc.scalar.dma_start(t[:], w2[dd])
                w2sb[dd] = t

            def body():
                for b, bw in enumerate(blocks):
                    c0 = b * BLK
                    if b == 0:
                        xgsb = xg0sb
                    else:
                        xgsb = []
                        for k in range(KD):
                            t = xgp.tile([P, BLK], bf16, tag=f"xg_{k}")
                            nc.sync.dma_start(t[:, :bw], xg[k, :, c0:c0 + bw])
                            xgsb.append(t)
                    hts = []
                    for fi in range(KF):
                        ps = ps1.tile([P, BLK], f32)
                        for k in range(KD):
                            nc.tensor.matmul(
                                ps[:, :bw], w1sb[fi][:, k * P:(k + 1) * P],
                                xgsb[k][:, :bw],
                                start=(k == 0), stop=(k == KD - 1))
                        ht = htp.tile([P, BLK], bf16, tag=f"ht_{fi}")
                        nc.vector.tensor_scalar_max(
                            ht[:, :bw], ps[:, :bw], 0.0)
                        hts.append(ht)
                    for d in range(KD):
                        ps_ = ps2.tile([P, BLK], f32)
                        for k2 in range(KF):
                            nc.tensor.matmul(
                                ps_[:, :bw],
                                w2sb[d][:, k2 * P:(k2 + 1) * P],
                                hts[k2][:, :bw],
                                start=(k2 == 0), stop=(k2 == KF - 1))
                        ob = outp.tile([P, BLK], bf16)
                        if d % 2 == 1:
                            nc.scalar.activation(
                                ob[:, :bw], ps_[:, :bw],
                                mybir.ActivationFunctionType.Copy)
                        else:
                            nc.vector.tensor_copy(ob[:, :bw], ps_[:, :bw])
                        nc.scalar.dma_start(ys[d, :, c0:c0 + bw], ob[:, :bw])

            if reps == 1:
                body()
            else:
                # Bench-only variant: repeat the whole body on-device so the
                # per-iteration time dominates host dispatch overhead.
                with tc.For_i(0, reps, 1):
                    body()
    nc.compile()
    return nc


def _get_compiled(cmax: int):
    reps = int(os.environ.get("KERNEL_REPS", "1"))
    key = (cmax, reps)
    if key not in _COMPILED:
        _COMPILED[key] = _build(cmax, reps)
    return _COMPILED[key]


def kernel(x, Wr, W1, W2, _timing=None):
    from concourse.bass_utils import run_bass_kernel_spmd

    x = np.asarray(x, dtype=np.float32)
    Wr = np.asarray(Wr, dtype=np.float32)
    W1 = np.asarray(W1, dtype=np.float32)
    W2 = np.asarray(W2, dtype=np.float32)
    xf = x.reshape(T, D)

    # --- Host router (replicated, reference-exact) => sharding plan ---
    idxs, gates = _route(xf, Wr)
    counts = [int(np.count_nonzero(gates[e])) for e in range(E)]
    cmax = min(CAP, max(P, max(math.ceil(c / P) * P for c in counts)))

    # --- Dispatch: gather routed rows per expert, fold gate, cast bf16 ---
    xfT = np.ascontiguousarray(xf.T)                       # [D, T] f32
    in_maps = []
    for e in range(E):
        n = counts[e]
        xge = np.zeros((D, cmax), dtype=np.float32)
        xge[:, :n] = xfT[:, idxs[e][:n]] * gates[e][:n][None, :]
        w1t = W1[e].reshape(KD, P, KF, P).transpose(2, 1, 0, 3)
        w2t = W2[e].reshape(KF, P, KD, P).transpose(2, 1, 0, 3)
        in_maps.append({
            "xg": np.ascontiguousarray(
                xge.reshape(KD, P, cmax)).astype(BF16),
            "w1": np.ascontiguousarray(w1t.reshape(KF, P, D)).astype(BF16),
            "w2": np.ascontiguousarray(w2t.reshape(KD, P, F)).astype(BF16),
        })

    # --- Device: 8 expert FFNs on 8 cores ---
    nc = _get_compiled(cmax)
    want_trace = _timing is not None and os.environ.get("KERNEL_TRACE", "1") == "1"
    tcores = [int(c) for c in os.environ.get(
        "KERNEL_TRACE_CORES", ",".join(map(str, range(NCORES)))).split(",")]
    try:
        res = run_bass_kernel_spmd(
            nc, in_maps, list(range(NCORES)),
            trace=want_trace,
            trace_cores=tcores if want_trace else None,
        )
    except ModuleNotFoundError:
        # NTFF profile hook unavailable in this environment: run untraced.
        res = run_bass_kernel_spmd(nc, in_maps, list(range(NCORES)))
    if _timing is not None:
        _timing["exec_time_ns"] = res.exec_time_ns
        _timing["results"] = res

    # --- Combine/unshard: scatter-add gated outputs into y (host, f32) ---
    y = np.zeros((T, D), dtype=np.float32)
    for e in range(E):
        n = counts[e]
        yse = res.results[e]["ys"]                         # [KD, P, cmax] bf16
        y[idxs[e][:n]] += yse.reshape(D, cmax)[:, :n].T.astype(np.float32)
    return y.reshape(B, S, D)
